# revision 19
# baseline (speedup 1.0000x reference)
# Trainium2 Bass kernel for DeltaPredictor (conv encoder -> GRU -> deconv decoder).
#
# Layout strategy (per core, batch-parallel over 8 cores, BC=64 batch each):
#   Everything on-device runs in "transposed" space: feature/hidden dims on SBUF
#   partitions, (batch, time) on the free axis. This keeps the GRU recurrence
#   transpose-free: each step's state update directly produces the next step's
#   matmul moving operand.
#
#   - hidden permutation h' = ij*32 + c (h = c*16 + ij) makes the decoder
#     block-diagonal at 128 granularity (4 diagonal matmuls).
#   - input-feature permutation f' = ij*32 + o makes the encoder block-diagonal
#     (per patch-row i, one K=64x2 matmul against pixel-major frames).
#   Both permutations are absorbed into host-side weight reshuffles.
#
#   Work is blocked over time in BLK=7 step blocks (T2=126=18*7): encoder+input
#   GEMM for block B+1 are emitted after the recurrence steps of block B so the
#   Tile scheduler fills recurrence dependency gaps on the PE with GEMM work.

import numpy as np

CH = 32
HID = 512
NCORES = 8
BLK = 7
BCAST_BC = 64  # per-core batch (512/8); used for host-side broadcast tiles

_cache = {}

# test instrumentation (harness uses defaults): set TRACE=True before calling
# kernel() to capture an NTFF profile; the result lands in LAST_RESULT
TRACE = False
LAST_RESULT = None


def _build(T, BC, sim=False):
    from contextlib import ExitStack

    import concourse.tile as tile
    from concourse import bacc, mybir

    f32 = mybir.dt.float32
    bf16 = mybir.dt.bfloat16
    AF = mybir.ActivationFunctionType
    OP = mybir.AluOpType

    T2 = T - 2
    NB = T2 // BLK
    assert NB * BLK == T2

    nc = bacc.Bacc("TRN2", target_bir_lowering=False)

    framesT = nc.dram_tensor("framesT", [2, 128, BC, T], f32, kind="ExternalInput")
    whh = nc.dram_tensor("whh", [128, 4, 12, 128], bf16, kind="ExternalInput")
    wih = nc.dram_tensor("wih", [128, 4, 12, 128], bf16, kind="ExternalInput")
    wenc = nc.dram_tensor("wenc", [128, 2, 128], bf16, kind="ExternalInput")
    wdec = nc.dram_tensor("wdec", [128, 64], bf16, kind="ExternalInput")
    encb = nc.dram_tensor("encb", [128, 1], f32, kind="ExternalInput")
    bcomb = nc.dram_tensor("bcomb", [128, 12], f32, kind="ExternalInput")
    bhhnbc = nc.dram_tensor("bhhnbc", [128, 4, BC], bf16, kind="ExternalInput")
    ident = nc.dram_tensor("ident", [128, 128], bf16, kind="ExternalInput")
    decb = nc.dram_tensor("decb", [128, 1], f32, kind="ExternalInput")
    outT = nc.dram_tensor("outT", [2, 128, BC, T2], f32, kind="ExternalOutput")

    with tile.TileContext(nc) as tc, ExitStack() as ctx:
        consts = ctx.enter_context(tc.tile_pool(name="consts", bufs=1))
        featp = ctx.enter_context(tc.tile_pool(name="featp", bufs=2))
        gxp = ctx.enter_context(tc.tile_pool(name="gxp", bufs=2))
        outsp = ctx.enter_context(tc.tile_pool(name="outsp", bufs=2))
        stepp = ctx.enter_context(tc.tile_pool(name="stepp", bufs=2))
        decp = ctx.enter_context(tc.tile_pool(name="decp", bufs=2))
        encp = ctx.enter_context(tc.tile_pool(name="encp", bufs=2))
        ps_gh = ctx.enter_context(tc.tile_pool(name="ps_gh", bufs=2, space="PSUM"))
        ps_gx = ctx.enter_context(tc.tile_pool(name="ps_gx", bufs=2, space="PSUM"))
        ps_enc = ctx.enter_context(tc.tile_pool(name="ps_enc", bufs=1, space="PSUM"))
        ps_dec = ctx.enter_context(tc.tile_pool(name="ps_dec", bufs=1, space="PSUM"))

        whh_sb = consts.tile([128, 4, 12, 128], bf16)
        nc.sync.dma_start(out=whh_sb[:], in_=whh[:])
        wih_sb = consts.tile([128, 4, 12, 128], bf16)
        nc.sync.dma_start(out=wih_sb[:], in_=wih[:])
        wenc_sb = consts.tile([128, 2, 128], bf16)
        nc.sync.dma_start(out=wenc_sb[:], in_=wenc[:])
        wdec_sb = consts.tile([128, 64], bf16)
        nc.sync.dma_start(out=wdec_sb[:], in_=wdec[:])
        encb_sb = consts.tile([128, 1], f32)
        nc.sync.dma_start(out=encb_sb[:], in_=encb[:])
        bcomb_sb = consts.tile([128, 12], f32)
        nc.sync.dma_start(out=bcomb_sb[:], in_=bcomb[:])
        bhhnbc_sb = consts.tile([128, 4, BC], bf16)
        nc.sync.dma_start(out=bhhnbc_sb[:], in_=bhhnbc[:])
        ident_sb = consts.tile([128, 128], bf16)
        nc.sync.dma_start(out=ident_sb[:], in_=ident[:])
        decb_sb = consts.tile([128, 1], f32)
        nc.sync.dma_start(out=decb_sb[:], in_=decb[:])

        # pixel-major frames, cast to bf16 on the way in (SWDGE cast DMA),
        # split into t-strips so early blocks start before the full load lands
        pixbf = []
        for h in range(2):
            pt = consts.tile([128, BC, T], bf16, name=f"pixbf{h}")
            pixbf.append(pt)
        nstrip = 4
        ts_ = T // nstrip
        for h in range(2):
            for s in range(nstrip):
                t0 = s * ts_
                nc.gpsimd.dma_start(
                    out=pixbf[h][:, :, t0 : t0 + ts_],
                    in_=framesT[h, :, :, t0 : t0 + ts_],
                )

        h0bf = consts.tile([128, 4, BC], bf16)
        nc.vector.memset(h0bf[:], 0.0)

        # Exact GELU via Erf so the whole kernel stays in the sigmoid/tanh/erf
        # activation table set (no per-block ~2.7us table reloads). The 0.5 of
        # gelu(x)=0.5*x*(1+erf(x/sqrt2)) is folded into w_ih host-side.
        # CoreSim has no Erf; sim mode substitutes Sigmoid (structure check only).
        erf_func = AF.Sigmoid if sim else AF.Erf

        def emit_enc(beta, featbuf):
            for i in range(4):
                pse = ps_enc.tile([128, BLK, BC], f32, name="pse")
                half = i % 2
                tilei = i // 2
                base = 64 * half
                for s in range(2):  # s=0: prev frame (t'), s=1: curr frame (t'+1)
                    t0 = BLK * beta + s
                    rhs = pixbf[tilei][base : base + 64, :, t0 : t0 + BLK]
                    rhs = rhs.transpose([0, 2, 1])  # free dims -> (t, b)
                    nc.tensor.matmul(
                        out=pse[:],
                        lhsT=wenc_sb[base : base + 64, s, :],
                        rhs=rhs,
                        start=(s == 0),
                        stop=(s == 1),
                    )
                xsb = encp.tile([128, BLK, BC], f32, name="xsb")
                nc.vector.tensor_scalar_add(out=xsb[:], in0=pse[:], scalar1=encb_sb[:, 0:1])
                erft = encp.tile([128, BLK, BC], f32, name="erft")
                nc.scalar.activation(
                    out=erft[:], in_=xsb[:], func=erf_func, scale=0.7071067811865476
                )
                nc.vector.scalar_tensor_tensor(
                    out=featbuf[:, i, :, :],
                    in0=erft[:],
                    scalar=1.0,
                    in1=xsb[:],
                    op0=OP.add,
                    op1=OP.mult,
                )

        def emit_gx(beta, featbuf, gxbuf):
            for m in range(12):
                psg = ps_gx.tile([128, BLK, BC], f32, name="psg")
                for k in range(4):
                    nc.tensor.matmul(
                        out=psg[:],
                        lhsT=wih_sb[:, k, m, :],
                        rhs=featbuf[:, k, :, :],
                        start=(k == 0),
                        stop=(k == 3),
                    )
                nc.scalar.activation(
                    out=gxbuf[:, m, :, :],
                    in_=psg[:],
                    func=AF.Identity,
                    bias=bcomb_sb[:, m : m + 1],
                    scale=1.0,
                )

        def emit_step(hbf, gxbuf, tt, outsbuf):
            # gh psum: m 0-7 (r,z) are preloaded with gx via identity matmuls so
            # sigmoid reads the finished sum straight from PSUM; m 8-11 (n) are
            # preloaded with b_hh_n (which sits inside the r* term).
            gh = ps_gh.tile([128, 12, BC], f32, name="gh")
            # one bank-wide start=True preload per PSUM bank (start clears
            # has_written for the WHOLE bank, so per-m-group preloads are
            # illegal); bank A (m 0-7, 512 f32) <- gx_rz, bank B (m 8-11) <- b_hh_n
            nc.tensor.matmul(
                out=gh[:, 0:8, :], lhsT=ident_sb[:], rhs=gxbuf[:, 0:8, tt, :],
                start=True, stop=False,
            )
            nc.tensor.matmul(
                out=gh[:, 8:12, :], lhsT=ident_sb[:], rhs=bhhnbc_sb[:],
                start=True, stop=False,
            )
            for m in range(12):
                for k in range(4):
                    # stop is sim-only bookkeeping; set it on the last matmul
                    # touching each bank
                    last_in_bank = (m == 7 or m == 11) and k == 3
                    nc.tensor.matmul(
                        out=gh[:, m, :],
                        lhsT=whh_sb[:, k, m, :],
                        rhs=hbf[:, k, :],
                        start=False,
                        stop=last_in_bank,
                    )
            rz = stepp.tile([128, 8, BC], f32, name="rz")
            nc.scalar.activation(out=rz[:], in_=gh[:, 0:8, :], func=AF.Sigmoid)
            # n = tanh(gx_n + r*(gh_n + b_hh_n));  psum n-part already holds gh_n+b_hh_n
            t2 = stepp.tile([128, 4, BC], f32, name="t2")
            nc.vector.tensor_mul(out=t2[:], in0=rz[:, 0:4, :], in1=gh[:, 8:12, :])
            npre = stepp.tile([128, 4, BC], f32, name="npre")
            nc.vector.tensor_add(out=npre[:], in0=t2[:], in1=gxbuf[:, 8:12, tt, :])
            nsb = stepp.tile([128, 4, BC], f32, name="nsb")
            nc.scalar.activation(out=nsb[:], in_=npre[:], func=AF.Tanh)
            # h' = n*(1-z) + z*h ; w=z*h and v=1-z run during the tanh window
            w = stepp.tile([128, 4, BC], f32, name="w")
            nc.vector.tensor_mul(out=w[:], in0=rz[:, 4:8, :], in1=hbf[:])
            v = stepp.tile([128, 4, BC], f32, name="v")
            nc.vector.tensor_scalar(
                out=v[:], in0=rz[:, 4:8, :], scalar1=-1.0, scalar2=1.0,
                op0=OP.mult, op1=OP.add,
            )
            u = stepp.tile([128, 4, BC], f32, name="u")
            nc.vector.tensor_mul(out=u[:], in0=nsb[:], in1=v[:])
            nc.vector.tensor_add(out=outsbuf[:, :, tt, :], in0=u[:], in1=w[:])

        def emit_dec(beta, outsbuf):
            for i2 in range(2):
                psd = ps_dec.tile([128, BLK, BC], f32, name="psd")
                for half in range(2):
                    i = i2 * 2 + half
                    nc.tensor.matmul(
                        out=psd[64 * half : 64 * half + 64, :, :],
                        lhsT=wdec_sb[:],
                        rhs=outsbuf[:, i, :, :],
                        start=True,
                        stop=True,
                    )
                curr = decp.tile([128, BC, BLK], f32, name="curr")
                nc.sync.dma_start(
                    out=curr[:],
                    in_=framesT[i2, :, :, BLK * beta + 1 : BLK * beta + 1 + BLK],
                )
                delta = decp.tile([128, BLK, BC], f32, name="delta")
                nc.scalar.activation(
                    out=delta[:], in_=psd[:], func=AF.Tanh, bias=decb_sb[:, 0:1]
                )
                # pred kept in (b, t) free layout so the output DMA APs balance
                pred = decp.tile([128, BC, BLK], f32, name="pred")
                nc.vector.tensor_add(
                    out=pred[:], in0=delta[:].transpose([0, 2, 1]), in1=curr[:]
                )
                nc.vector.tensor_scalar(
                    out=pred[:],
                    in0=pred[:],
                    scalar1=0.0,
                    scalar2=1.0,
                    op0=OP.max,
                    op1=OP.min,
                )
                nc.sync.dma_start(
                    out=outT[i2, :, :, BLK * beta : BLK * beta + BLK],
                    in_=pred[:],
                )

        featbuf = featp.tile([128, 4, BLK, BC], bf16, name="featbuf")
        emit_enc(0, featbuf)
        gxbuf = gxp.tile([128, 12, BLK, BC], bf16, name="gxbuf")
        emit_gx(0, featbuf, gxbuf)

        hbf = h0bf
        for beta in range(NB):
            cur_gx = gxbuf
            outsbuf = outsp.tile([128, 4, BLK, BC], bf16, name="outsbuf")
            for tt in range(BLK):
                emit_step(hbf, cur_gx, tt, outsbuf)
                hbf = outsbuf[:, :, tt, :]
            if beta + 1 < NB:
                featbuf = featp.tile([128, 4, BLK, BC], bf16, name="featbuf")
                emit_enc(beta + 1, featbuf)
                gxbuf = gxp.tile([128, 12, BLK, BC], bf16, name="gxbuf")
                emit_gx(beta + 1, featbuf, gxbuf)
            emit_dec(beta, outsbuf)

    nc.compile()
    return nc


def _build_null(T, BC):
    """Same I/O signature, trivial body — for differential wall-clock timing."""
    from contextlib import ExitStack

    import concourse.tile as tile
    from concourse import bacc, mybir

    f32 = mybir.dt.float32
    bf16 = mybir.dt.bfloat16
    T2 = T - 2
    nc = bacc.Bacc("TRN2", target_bir_lowering=False)
    nc.dram_tensor("framesT", [2, 128, BC, T], f32, kind="ExternalInput")
    nc.dram_tensor("whh", [128, 4, 12, 128], bf16, kind="ExternalInput")
    nc.dram_tensor("wih", [128, 4, 12, 128], bf16, kind="ExternalInput")
    nc.dram_tensor("wenc", [128, 2, 128], bf16, kind="ExternalInput")
    nc.dram_tensor("wdec", [128, 64], bf16, kind="ExternalInput")
    encb = nc.dram_tensor("encb", [128, 1], f32, kind="ExternalInput")
    nc.dram_tensor("bcomb", [128, 12], f32, kind="ExternalInput")
    nc.dram_tensor("bhhnbc", [128, 4, BC], bf16, kind="ExternalInput")
    nc.dram_tensor("ident", [128, 128], bf16, kind="ExternalInput")
    nc.dram_tensor("decb", [128, 1], f32, kind="ExternalInput")
    outT = nc.dram_tensor("outT", [2, 128, BC, T2], f32, kind="ExternalOutput")
    with tile.TileContext(nc) as tc, ExitStack() as ctx:
        pool = ctx.enter_context(tc.tile_pool(name="p", bufs=1))
        t = pool.tile([128, 1], f32)
        nc.sync.dma_start(out=t[:], in_=encb[:])
        nc.sync.dma_start(out=outT[0, :, 0, 0:1], in_=t[:])
    nc.compile()
    return nc


def _prep_weights(conv_w, conv_b, w_ih, w_hh, b_ih, b_hh, deconv_w, deconv_b):
    """Host-side weight reshuffles into the kernel's permuted/tiled layouts."""
    bf16 = np.dtype("bfloat16") if hasattr(np, "bfloat16") else None
    import ml_dtypes

    bf = ml_dtypes.bfloat16

    idx = np.arange(HID)
    hmap = (idx % 32) * 16 + (idx // 32)  # h' -> h  (h' = ij*32 + c)

    # 0.5 from the erf-form GELU is folded into w_ih (feat' = 2*gelu(x))
    w_ih2 = 0.5 * w_ih.reshape(3, HID, HID)[:, hmap, :][:, :, hmap].reshape(
        3 * HID, HID
    )
    w_hh2 = w_hh.reshape(3, HID, HID)[:, hmap, :][:, :, hmap].reshape(3 * HID, HID)
    b_ih2 = b_ih.reshape(3, HID)[:, hmap].reshape(3 * HID)
    b_hh2 = b_hh.reshape(3, HID)[:, hmap].reshape(3 * HID)

    # (kk, k, m, mm): lhsT(k,m)[kk,mm] = W2[m*128+mm, k*128+kk]
    whh_t = np.ascontiguousarray(
        w_hh2.T.reshape(4, 128, 12, 128).transpose(1, 0, 2, 3)
    ).astype(bf)
    wih_t = np.ascontiguousarray(
        w_ih2.T.reshape(4, 128, 12, 128).transpose(1, 0, 2, 3)
    ).astype(bf)

    # encoder: rows u=16p+4j+q in [0,64), cols j2*32+o; s=0 prev(c=1), s=1 curr(c=0)
    wenc_h = np.zeros((128, 2, 128), np.float32)
    u = np.arange(64)
    p, j, q = (u >> 4) & 3, (u >> 2) & 3, u & 3
    for s, c in ((0, 1), (1, 0)):
        blockw = np.zeros((64, 128), np.float32)
        for j2 in range(4):
            mask = j == j2
            # cols j2*32 + o ; rows where j(u)==j2 get conv_w[o, c, p(u), q(u)]
            blockw[mask, j2 * 32 : j2 * 32 + 32] = conv_w[:, c, p[mask], q[mask]].T
        wenc_h[0:64, s, :] = blockw
        wenc_h[64:128, s, :] = blockw
    wenc_t = wenc_h.astype(bf)

    # decoder: rows j*32+c, cols u2 = p*16 + j2*4 + q
    wdec_h = np.zeros((128, 64), np.float32)
    for jj in range(4):
        for c in range(CH):
            for pp in range(4):
                for qq in range(4):
                    wdec_h[jj * 32 + c, pp * 16 + jj * 4 + qq] = deconv_w[c, 0, pp, qq]
    wdec_t = wdec_h.astype(bf)

    encb_h = np.zeros((128, 1), np.float32)
    encb_h[:, 0] = conv_b[np.arange(128) % 32]

    bcomb_h = np.zeros((128, 12), np.float32)
    for m in range(12):
        g = m * 128 + np.arange(128)
        bcomb_h[:, m] = b_ih2[g] + (b_hh2[g] if m < 8 else 0.0)

    bhhn_h = np.zeros((128, 4), np.float32)
    for c in range(4):
        bhhn_h[:, c] = b_hh2[1024 + c * 128 + np.arange(128)]
    bhhnbc_h = np.repeat(bhhn_h[:, :, None], BCAST_BC, axis=2).astype(bf)

    ident_h = np.eye(128, dtype=np.float32).astype(bf)

    decb_h = np.full((128, 1), float(deconv_b[0]), np.float32)

    return dict(
        whh=whh_t,
        wih=wih_t,
        wenc=wenc_t,
        wdec=wdec_t,
        encb=encb_h,
        bcomb=bcomb_h,
        bhhnbc=bhhnbc_h,
        ident=ident_h,
        decb=decb_h,
    )


def kernel(frames, conv_w, conv_b, w_ih, w_hh, b_ih, b_hh, deconv_w, deconv_b):
    from concourse.bass_utils import run_bass_kernel_spmd

    frames = np.asarray(frames, np.float32)
    conv_w = np.asarray(conv_w, np.float32)
    conv_b = np.asarray(conv_b, np.float32)
    w_ih = np.asarray(w_ih, np.float32)
    w_hh = np.asarray(w_hh, np.float32)
    b_ih = np.asarray(b_ih, np.float32)
    b_hh = np.asarray(b_hh, np.float32)
    deconv_w = np.asarray(deconv_w, np.float32)
    deconv_b = np.asarray(deconv_b, np.float32)

    B, T = frames.shape[0], frames.shape[1]
    T2 = T - 2
    BC = B // NCORES

    key = (T, BC)
    if key not in _cache:
        _cache[key] = _build(T, BC)
    nc = _cache[key]

    wmap = _prep_weights(conv_w, conv_b, w_ih, w_hh, b_ih, b_hh, deconv_w, deconv_b)

    fr = frames.reshape(B, T, 256)
    in_maps = []
    for c in range(NCORES):
        sl = fr[c * BC : (c + 1) * BC]  # (BC, T, 256)
        framesT_c = np.ascontiguousarray(sl.transpose(2, 0, 1)).reshape(2, 128, BC, T)
        m = dict(wmap)
        m["framesT"] = framesT_c
        in_maps.append(m)

    res = run_bass_kernel_spmd(
        nc, in_maps, core_ids=list(range(NCORES)), trace=TRACE
    )
    global LAST_RESULT
    LAST_RESULT = res

    out = np.empty((B, T2, 1, 16, 16), np.float32)
    for c in range(NCORES):
        o = res.results[c]["outT"]  # (2, 128, BC, T2)
        o = o.reshape(256, BC, T2).transpose(1, 2, 0).reshape(BC, T2, 1, 16, 16)
        out[c * BC : (c + 1) * BC] = o
    return out


# revision 21
# speedup vs baseline: 6.1037x; 6.1037x over previous
# Trainium2 Bass kernel for DeltaPredictor (conv encoder -> GRU -> deconv decoder).
#
# Layout strategy (per core, batch-parallel over 8 cores, BC=64 batch each):
#   Everything on-device runs in "transposed" space: feature/hidden dims on SBUF
#   partitions, (batch, time) on the free axis. This keeps the GRU recurrence
#   transpose-free: each step's state update directly produces the next step's
#   matmul moving operand.
#
#   - hidden permutation h' = ij*32 + c (h = c*16 + ij) makes the decoder
#     block-diagonal at 128 granularity (4 diagonal matmuls).
#   - input-feature permutation f' = ij*32 + o makes the encoder block-diagonal
#     (per patch-row i, one K=64x2 matmul against pixel-major frames).
#   Both permutations are absorbed into host-side weight reshuffles.
#
#   Work is blocked over time in BLK=7 step blocks (T2=126=18*7): encoder+input
#   GEMM for block B+1 are emitted after the recurrence steps of block B so the
#   Tile scheduler fills recurrence dependency gaps on the PE with GEMM work.

import numpy as np

CH = 32
HID = 512
NCORES = 8
BLK = 7
BCAST_BC = 64  # per-core batch (512/8); used for host-side broadcast tiles

_cache = {}

# test instrumentation (harness uses defaults): set TRACE=True before calling
# kernel() to capture an NTFF profile; the result lands in LAST_RESULT
TRACE = False
LAST_RESULT = None


def _build(T, BC, sim=False, reps=1):
    from contextlib import ExitStack

    import concourse.tile as tile
    from concourse import bacc, mybir

    f32 = mybir.dt.float32
    bf16 = mybir.dt.bfloat16
    AF = mybir.ActivationFunctionType
    OP = mybir.AluOpType

    T2 = T - 2
    NB = T2 // BLK
    assert NB * BLK == T2

    nc = bacc.Bacc("TRN2", target_bir_lowering=False)

    framesT = nc.dram_tensor("framesT", [2, 128, BC, T], f32, kind="ExternalInput")
    whh = nc.dram_tensor("whh", [128, 4, 12, 128], bf16, kind="ExternalInput")
    wih = nc.dram_tensor("wih", [128, 4, 12, 128], bf16, kind="ExternalInput")
    wenc = nc.dram_tensor("wenc", [128, 2, 128], bf16, kind="ExternalInput")
    wdec = nc.dram_tensor("wdec", [128, 64], bf16, kind="ExternalInput")
    encb = nc.dram_tensor("encb", [128, 1], f32, kind="ExternalInput")
    bcomb = nc.dram_tensor("bcomb", [128, 12], f32, kind="ExternalInput")
    bhhnbc = nc.dram_tensor("bhhnbc", [128, 4, BC], bf16, kind="ExternalInput")
    ident = nc.dram_tensor("ident", [128, 128], bf16, kind="ExternalInput")
    decb = nc.dram_tensor("decb", [128, 1], f32, kind="ExternalInput")
    outT = nc.dram_tensor("outT", [2, 128, BC, T2], f32, kind="ExternalOutput")

    with tile.TileContext(nc) as tc, ExitStack() as ctx:
        consts = ctx.enter_context(tc.tile_pool(name="consts", bufs=1))
        featp = ctx.enter_context(tc.tile_pool(name="featp", bufs=2))
        gxp = ctx.enter_context(tc.tile_pool(name="gxp", bufs=2))
        outsp = ctx.enter_context(tc.tile_pool(name="outsp", bufs=2))
        stepp = ctx.enter_context(tc.tile_pool(name="stepp", bufs=2))
        decp = ctx.enter_context(tc.tile_pool(name="decp", bufs=2))
        encp = ctx.enter_context(tc.tile_pool(name="encp", bufs=2))
        ps_gh = ctx.enter_context(tc.tile_pool(name="ps_gh", bufs=2, space="PSUM"))
        ps_gx = ctx.enter_context(tc.tile_pool(name="ps_gx", bufs=2, space="PSUM"))
        ps_enc = ctx.enter_context(tc.tile_pool(name="ps_enc", bufs=1, space="PSUM"))
        ps_dec = ctx.enter_context(tc.tile_pool(name="ps_dec", bufs=1, space="PSUM"))

        whh_sb = consts.tile([128, 4, 12, 128], bf16)
        nc.sync.dma_start(out=whh_sb[:], in_=whh[:])
        wih_sb = consts.tile([128, 4, 12, 128], bf16)
        nc.sync.dma_start(out=wih_sb[:], in_=wih[:])
        wenc_sb = consts.tile([128, 2, 128], bf16)
        nc.sync.dma_start(out=wenc_sb[:], in_=wenc[:])
        wdec_sb = consts.tile([128, 64], bf16)
        nc.sync.dma_start(out=wdec_sb[:], in_=wdec[:])
        encb_sb = consts.tile([128, 1], f32)
        nc.sync.dma_start(out=encb_sb[:], in_=encb[:])
        bcomb_sb = consts.tile([128, 12], f32)
        nc.sync.dma_start(out=bcomb_sb[:], in_=bcomb[:])
        bhhnbc_sb = consts.tile([128, 4, BC], bf16)
        nc.sync.dma_start(out=bhhnbc_sb[:], in_=bhhnbc[:])
        ident_sb = consts.tile([128, 128], bf16)
        nc.sync.dma_start(out=ident_sb[:], in_=ident[:])
        decb_sb = consts.tile([128, 1], f32)
        nc.sync.dma_start(out=decb_sb[:], in_=decb[:])

        # pixel-major frames, cast to bf16 on the way in (SWDGE cast DMA),
        # split into t-strips so early blocks start before the full load lands
        pixbf = []
        for h in range(2):
            pt = consts.tile([128, BC, T], bf16, name=f"pixbf{h}")
            pixbf.append(pt)
        nstrip = 4
        ts_ = T // nstrip
        for h in range(2):
            for s in range(nstrip):
                t0 = s * ts_
                nc.gpsimd.dma_start(
                    out=pixbf[h][:, :, t0 : t0 + ts_],
                    in_=framesT[h, :, :, t0 : t0 + ts_],
                )

        h0bf = consts.tile([128, 4, BC], bf16)
        nc.vector.memset(h0bf[:], 0.0)

        # Exact GELU via Erf so the whole kernel stays in the sigmoid/tanh/erf
        # activation table set (no per-block ~2.7us table reloads). The 0.5 of
        # gelu(x)=0.5*x*(1+erf(x/sqrt2)) is folded into w_ih host-side.
        # CoreSim has no Erf; sim mode substitutes Sigmoid (structure check only).
        erf_func = AF.Sigmoid if sim else AF.Erf

        def emit_enc(beta, featbuf):
            for i in range(4):
                pse = ps_enc.tile([128, BLK, BC], f32, name="pse")
                half = i % 2
                tilei = i // 2
                base = 64 * half
                for s in range(2):  # s=0: prev frame (t'), s=1: curr frame (t'+1)
                    t0 = BLK * beta + s
                    rhs = pixbf[tilei][base : base + 64, :, t0 : t0 + BLK]
                    rhs = rhs.transpose([0, 2, 1])  # free dims -> (t, b)
                    nc.tensor.matmul(
                        out=pse[:],
                        lhsT=wenc_sb[base : base + 64, s, :],
                        rhs=rhs,
                        start=(s == 0),
                        stop=(s == 1),
                    )
                xsb = encp.tile([128, BLK, BC], f32, name="xsb")
                nc.vector.tensor_scalar_add(out=xsb[:], in0=pse[:], scalar1=encb_sb[:, 0:1])
                erft = encp.tile([128, BLK, BC], f32, name="erft")
                nc.scalar.activation(
                    out=erft[:], in_=xsb[:], func=erf_func, scale=0.7071067811865476
                )
                nc.vector.scalar_tensor_tensor(
                    out=featbuf[:, i, :, :],
                    in0=erft[:],
                    scalar=1.0,
                    in1=xsb[:],
                    op0=OP.add,
                    op1=OP.mult,
                )

        def emit_gx(beta, featbuf, gxbuf):
            for m in range(12):
                psg = ps_gx.tile([128, BLK, BC], f32, name="psg")
                for k in range(4):
                    nc.tensor.matmul(
                        out=psg[:],
                        lhsT=wih_sb[:, k, m, :],
                        rhs=featbuf[:, k, :, :],
                        start=(k == 0),
                        stop=(k == 3),
                    )
                nc.scalar.activation(
                    out=gxbuf[:, m, :, :],
                    in_=psg[:],
                    func=AF.Identity,
                    bias=bcomb_sb[:, m : m + 1],
                    scale=1.0,
                )

        def emit_step(hbf, gxbuf, tt, outsbuf):
            # gh psum: m 0-7 (r,z) are preloaded with gx via identity matmuls so
            # sigmoid reads the finished sum straight from PSUM; m 8-11 (n) are
            # preloaded with b_hh_n (which sits inside the r* term).
            gh = ps_gh.tile([128, 12, BC], f32, name="gh")
            # one bank-wide start=True preload per PSUM bank (start clears
            # has_written for the WHOLE bank, so per-m-group preloads are
            # illegal); bank A (m 0-7, 512 f32) <- gx_rz, bank B (m 8-11) <- b_hh_n
            nc.tensor.matmul(
                out=gh[:, 0:8, :], lhsT=ident_sb[:], rhs=gxbuf[:, 0:8, tt, :],
                start=True, stop=False,
            )
            nc.tensor.matmul(
                out=gh[:, 8:12, :], lhsT=ident_sb[:], rhs=bhhnbc_sb[:],
                start=True, stop=False,
            )
            for m in range(12):
                for k in range(4):
                    # stop is sim-only bookkeeping; set it on the last matmul
                    # touching each bank
                    last_in_bank = (m == 7 or m == 11) and k == 3
                    nc.tensor.matmul(
                        out=gh[:, m, :],
                        lhsT=whh_sb[:, k, m, :],
                        rhs=hbf[:, k, :],
                        start=False,
                        stop=last_in_bank,
                    )
            rz = stepp.tile([128, 8, BC], f32, name="rz")
            nc.scalar.activation(out=rz[:], in_=gh[:, 0:8, :], func=AF.Sigmoid)
            # n = tanh(gx_n + r*(gh_n + b_hh_n));  psum n-part already holds gh_n+b_hh_n
            t2 = stepp.tile([128, 4, BC], f32, name="t2")
            nc.vector.tensor_mul(out=t2[:], in0=rz[:, 0:4, :], in1=gh[:, 8:12, :])
            npre = stepp.tile([128, 4, BC], f32, name="npre")
            nc.vector.tensor_add(out=npre[:], in0=t2[:], in1=gxbuf[:, 8:12, tt, :])
            nsb = stepp.tile([128, 4, BC], f32, name="nsb")
            nc.scalar.activation(out=nsb[:], in_=npre[:], func=AF.Tanh)
            # h' = n*(1-z) + z*h ; w=z*h and v=1-z run during the tanh window
            w = stepp.tile([128, 4, BC], f32, name="w")
            nc.vector.tensor_mul(out=w[:], in0=rz[:, 4:8, :], in1=hbf[:])
            v = stepp.tile([128, 4, BC], f32, name="v")
            nc.vector.tensor_scalar(
                out=v[:], in0=rz[:, 4:8, :], scalar1=-1.0, scalar2=1.0,
                op0=OP.mult, op1=OP.add,
            )
            u = stepp.tile([128, 4, BC], f32, name="u")
            nc.vector.tensor_mul(out=u[:], in0=nsb[:], in1=v[:])
            nc.vector.tensor_add(out=outsbuf[:, :, tt, :], in0=u[:], in1=w[:])

        def emit_dec(beta, outsbuf):
            for i2 in range(2):
                psd = ps_dec.tile([128, BLK, BC], f32, name="psd")
                for half in range(2):
                    i = i2 * 2 + half
                    nc.tensor.matmul(
                        out=psd[64 * half : 64 * half + 64, :, :],
                        lhsT=wdec_sb[:],
                        rhs=outsbuf[:, i, :, :],
                        start=True,
                        stop=True,
                    )
                curr = decp.tile([128, BC, BLK], f32, name="curr")
                nc.sync.dma_start(
                    out=curr[:],
                    in_=framesT[i2, :, :, BLK * beta + 1 : BLK * beta + 1 + BLK],
                )
                delta = decp.tile([128, BLK, BC], f32, name="delta")
                nc.scalar.activation(
                    out=delta[:], in_=psd[:], func=AF.Tanh, bias=decb_sb[:, 0:1]
                )
                # pred kept in (b, t) free layout so the output DMA APs balance
                pred = decp.tile([128, BC, BLK], f32, name="pred")
                nc.vector.tensor_add(
                    out=pred[:], in0=delta[:].transpose([0, 2, 1]), in1=curr[:]
                )
                nc.vector.tensor_scalar(
                    out=pred[:],
                    in0=pred[:],
                    scalar1=0.0,
                    scalar2=1.0,
                    op0=OP.max,
                    op1=OP.min,
                )
                nc.sync.dma_start(
                    out=outT[i2, :, :, BLK * beta : BLK * beta + BLK],
                    in_=pred[:],
                )

        def emit_pipeline():
            featbuf = featp.tile([128, 4, BLK, BC], bf16, name="featbuf")
            emit_enc(0, featbuf)
            gxbuf = gxp.tile([128, 12, BLK, BC], bf16, name="gxbuf")
            emit_gx(0, featbuf, gxbuf)

            hbf = h0bf
            for beta in range(NB):
                cur_gx = gxbuf
                outsbuf = outsp.tile([128, 4, BLK, BC], bf16, name="outsbuf")
                for tt in range(BLK):
                    emit_step(hbf, cur_gx, tt, outsbuf)
                    hbf = outsbuf[:, :, tt, :]
                if beta + 1 < NB:
                    featbuf = featp.tile([128, 4, BLK, BC], bf16, name="featbuf")
                    emit_enc(beta + 1, featbuf)
                    gxbuf = gxp.tile([128, 12, BLK, BC], bf16, name="gxbuf")
                    emit_gx(beta + 1, featbuf, gxbuf)
                emit_dec(beta, outsbuf)

        if reps == 1:
            emit_pipeline()
        else:
            # benchmarking only: re-run the whole (idempotent) body on-device
            with tc.For_i(0, reps, 1):
                emit_pipeline()

    nc.compile()
    return nc


def _build_null(T, BC):
    """Same I/O signature, trivial body — for differential wall-clock timing."""
    from contextlib import ExitStack

    import concourse.tile as tile
    from concourse import bacc, mybir

    f32 = mybir.dt.float32
    bf16 = mybir.dt.bfloat16
    T2 = T - 2
    nc = bacc.Bacc("TRN2", target_bir_lowering=False)
    nc.dram_tensor("framesT", [2, 128, BC, T], f32, kind="ExternalInput")
    nc.dram_tensor("whh", [128, 4, 12, 128], bf16, kind="ExternalInput")
    nc.dram_tensor("wih", [128, 4, 12, 128], bf16, kind="ExternalInput")
    nc.dram_tensor("wenc", [128, 2, 128], bf16, kind="ExternalInput")
    nc.dram_tensor("wdec", [128, 64], bf16, kind="ExternalInput")
    encb = nc.dram_tensor("encb", [128, 1], f32, kind="ExternalInput")
    nc.dram_tensor("bcomb", [128, 12], f32, kind="ExternalInput")
    nc.dram_tensor("bhhnbc", [128, 4, BC], bf16, kind="ExternalInput")
    nc.dram_tensor("ident", [128, 128], bf16, kind="ExternalInput")
    nc.dram_tensor("decb", [128, 1], f32, kind="ExternalInput")
    outT = nc.dram_tensor("outT", [2, 128, BC, T2], f32, kind="ExternalOutput")
    with tile.TileContext(nc) as tc, ExitStack() as ctx:
        pool = ctx.enter_context(tc.tile_pool(name="p", bufs=1))
        t = pool.tile([128, 1], f32)
        nc.sync.dma_start(out=t[:], in_=encb[:])
        nc.sync.dma_start(out=outT[0, :, 0, 0:1], in_=t[:])
    nc.compile()
    return nc


def _prep_weights(conv_w, conv_b, w_ih, w_hh, b_ih, b_hh, deconv_w, deconv_b):
    """Host-side weight reshuffles into the kernel's permuted/tiled layouts."""
    bf16 = np.dtype("bfloat16") if hasattr(np, "bfloat16") else None
    import ml_dtypes

    bf = ml_dtypes.bfloat16

    idx = np.arange(HID)
    hmap = (idx % 32) * 16 + (idx // 32)  # h' -> h  (h' = ij*32 + c)

    # 0.5 from the erf-form GELU is folded into w_ih (feat' = 2*gelu(x))
    w_ih2 = 0.5 * w_ih.reshape(3, HID, HID)[:, hmap, :][:, :, hmap].reshape(
        3 * HID, HID
    )
    w_hh2 = w_hh.reshape(3, HID, HID)[:, hmap, :][:, :, hmap].reshape(3 * HID, HID)
    b_ih2 = b_ih.reshape(3, HID)[:, hmap].reshape(3 * HID)
    b_hh2 = b_hh.reshape(3, HID)[:, hmap].reshape(3 * HID)

    # (kk, k, m, mm): lhsT(k,m)[kk,mm] = W2[m*128+mm, k*128+kk]
    whh_t = np.ascontiguousarray(
        w_hh2.T.reshape(4, 128, 12, 128).transpose(1, 0, 2, 3)
    ).astype(bf)
    wih_t = np.ascontiguousarray(
        w_ih2.T.reshape(4, 128, 12, 128).transpose(1, 0, 2, 3)
    ).astype(bf)

    # encoder: rows u=16p+4j+q in [0,64), cols j2*32+o; s=0 prev(c=1), s=1 curr(c=0)
    wenc_h = np.zeros((128, 2, 128), np.float32)
    u = np.arange(64)
    p, j, q = (u >> 4) & 3, (u >> 2) & 3, u & 3
    for s, c in ((0, 1), (1, 0)):
        blockw = np.zeros((64, 128), np.float32)
        for j2 in range(4):
            mask = j == j2
            # cols j2*32 + o ; rows where j(u)==j2 get conv_w[o, c, p(u), q(u)]
            blockw[mask, j2 * 32 : j2 * 32 + 32] = conv_w[:, c, p[mask], q[mask]].T
        wenc_h[0:64, s, :] = blockw
        wenc_h[64:128, s, :] = blockw
    wenc_t = wenc_h.astype(bf)

    # decoder: rows j*32+c, cols u2 = p*16 + j2*4 + q
    wdec_h = np.zeros((128, 64), np.float32)
    for jj in range(4):
        for c in range(CH):
            for pp in range(4):
                for qq in range(4):
                    wdec_h[jj * 32 + c, pp * 16 + jj * 4 + qq] = deconv_w[c, 0, pp, qq]
    wdec_t = wdec_h.astype(bf)

    encb_h = np.zeros((128, 1), np.float32)
    encb_h[:, 0] = conv_b[np.arange(128) % 32]

    bcomb_h = np.zeros((128, 12), np.float32)
    for m in range(12):
        g = m * 128 + np.arange(128)
        bcomb_h[:, m] = b_ih2[g] + (b_hh2[g] if m < 8 else 0.0)

    bhhn_h = np.zeros((128, 4), np.float32)
    for c in range(4):
        bhhn_h[:, c] = b_hh2[1024 + c * 128 + np.arange(128)]
    bhhnbc_h = np.repeat(bhhn_h[:, :, None], BCAST_BC, axis=2).astype(bf)

    ident_h = np.eye(128, dtype=np.float32).astype(bf)

    decb_h = np.full((128, 1), float(deconv_b[0]), np.float32)

    return dict(
        whh=whh_t,
        wih=wih_t,
        wenc=wenc_t,
        wdec=wdec_t,
        encb=encb_h,
        bcomb=bcomb_h,
        bhhnbc=bhhnbc_h,
        ident=ident_h,
        decb=decb_h,
    )


def kernel(frames, conv_w, conv_b, w_ih, w_hh, b_ih, b_hh, deconv_w, deconv_b):
    from concourse.bass_utils import run_bass_kernel_spmd

    frames = np.asarray(frames, np.float32)
    conv_w = np.asarray(conv_w, np.float32)
    conv_b = np.asarray(conv_b, np.float32)
    w_ih = np.asarray(w_ih, np.float32)
    w_hh = np.asarray(w_hh, np.float32)
    b_ih = np.asarray(b_ih, np.float32)
    b_hh = np.asarray(b_hh, np.float32)
    deconv_w = np.asarray(deconv_w, np.float32)
    deconv_b = np.asarray(deconv_b, np.float32)

    B, T = frames.shape[0], frames.shape[1]
    T2 = T - 2
    BC = B // NCORES

    key = (T, BC)
    if key not in _cache:
        _cache[key] = _build(T, BC)
    nc = _cache[key]

    wmap = _prep_weights(conv_w, conv_b, w_ih, w_hh, b_ih, b_hh, deconv_w, deconv_b)

    fr = frames.reshape(B, T, 256)
    in_maps = []
    for c in range(NCORES):
        sl = fr[c * BC : (c + 1) * BC]  # (BC, T, 256)
        framesT_c = np.ascontiguousarray(sl.transpose(2, 0, 1)).reshape(2, 128, BC, T)
        m = dict(wmap)
        m["framesT"] = framesT_c
        in_maps.append(m)

    res = run_bass_kernel_spmd(
        nc, in_maps, core_ids=list(range(NCORES)), trace=TRACE
    )
    global LAST_RESULT
    LAST_RESULT = res

    out = np.empty((B, T2, 1, 16, 16), np.float32)
    for c in range(NCORES):
        o = res.results[c]["outT"]  # (2, 128, BC, T2)
        o = o.reshape(256, BC, T2).transpose(1, 2, 0).reshape(BC, T2, 1, 16, 16)
        out[c * BC : (c + 1) * BC] = o
    return out
